# revision 7
# baseline (speedup 1.0000x reference)
"""Trainium2 Bass kernel for ConvspatialAttentionBlock.

Computes, per batch b:
  q = Wq @ x + bq            [64, N]
  k = Wk @ x + bk            [64, N]
  v = Wv @ x + bv            [512, N]
  P = softmax(q^T k, axis=j) [N, N]
  out = gamma * (v @ P^T) + x

Sharding: 8 cores = (batch b in 0..3) x (query-half h in 0..1). Each core
computes attention output for its 2048 query positions against all 4096
keys of its batch. Host rolls the input columns so each core's queries are
always columns 0:2048 of its x (key order is irrelevant to softmax+AV).

Device algebra (per core), all matmuls in float32r (full PE rate, ~1.5e-4):
  gamma and bv are folded host-side: Wv' = gamma*Wv, bv' = gamma*bv, so
  out = (sum_j v'_raw[c,j] e[j,i]) / den[i] + bv'[c] + x[c,i]
  where e = exp(logits^T) (no max subtraction needed: |logits| <~ 10),
  den[i] = sum_j e[j,i] accumulated on the PE via ones-vector matmuls.
"""

import numpy as np

import concourse.bacc as bacc
import concourse.mybir as mybir
import concourse.tile as tile

B, C, N = 4, 512, 4096
D = 64            # query/key channels (C//8)
NQ = N // 2       # queries per core
NCORES = 8
IC = 512          # query-chunk (free dim per matmul)
NIC = NQ // IC    # 4 query chunks
NJT = N // 128    # 32 key tiles
CCH = C // 128    # 4 channel chunks

F32 = mybir.dt.float32
F32R = mybir.dt.float32r
ACT_COPY = mybir.ActivationFunctionType.Copy
ACT_EXP = mybir.ActivationFunctionType.Exp
ACT_IDENT = mybir.ActivationFunctionType.Identity


def build():
    nc = bacc.Bacc("TRN2", target_bir_lowering=False, debug=False,
                   num_devices=NCORES)

    x_d = nc.dram_tensor("x", [C, N], F32R, kind="ExternalInput")
    wqT_d = nc.dram_tensor("wqT", [C, D], F32R, kind="ExternalInput")
    wkT_d = nc.dram_tensor("wkT", [C, D], F32R, kind="ExternalInput")
    wvT_d = nc.dram_tensor("wvT", [C, C], F32R, kind="ExternalInput")
    bq_d = nc.dram_tensor("bq", [D, 1], F32, kind="ExternalInput")
    bk_d = nc.dram_tensor("bk", [D, 1], F32, kind="ExternalInput")
    bvs_d = nc.dram_tensor("bvs", [C, 1], F32, kind="ExternalInput")
    onesc_d = nc.dram_tensor("onesc", [128, 1], F32R, kind="ExternalInput")
    onesr_d = nc.dram_tensor("onesr", [1, 128], F32R, kind="ExternalInput")
    out_d = nc.dram_tensor("out", [C, NQ], F32, kind="ExternalOutput")

    with tile.TileContext(nc) as tc:
        with (
            tc.tile_pool(name="persist", bufs=1) as pp,
            tc.tile_pool(name="work", bufs=3) as wp,
            tc.tile_pool(name="fin", bufs=2) as fp,
            tc.tile_pool(name="ps2", bufs=2, space="PSUM") as ps2,
            tc.tile_pool(name="ps1", bufs=1, space="PSUM") as ps1,
        ):
            # ---- persistent SBUF ----
            x_t = [pp.tile([128, N], F32R, tag=f"x{i}", name=f"x{i}") for i in range(CCH)]
            for i in range(CCH):
                nc.sync.dma_start(x_t[i][:], x_d.ap()[i * 128:(i + 1) * 128, :])
            wq_t = pp.tile([128, CCH, D], F32R, tag="wq")
            nc.sync.dma_start(
                wq_t[:], wqT_d.ap().rearrange("(a p) d -> p a d", p=128))
            wk_t = pp.tile([128, CCH, D], F32R, tag="wk")
            nc.sync.dma_start(
                wk_t[:], wkT_d.ap().rearrange("(a p) d -> p a d", p=128))
            wv_t = pp.tile([128, CCH, C], F32R, tag="wv")
            nc.sync.dma_start(
                wv_t[:], wvT_d.ap().rearrange("(a p) c -> p a c", p=128))
            bq_t = pp.tile([D, 1], F32, tag="bq")
            nc.sync.dma_start(bq_t[:], bq_d.ap())
            bk_t = pp.tile([D, 1], F32, tag="bk")
            nc.sync.dma_start(bk_t[:], bk_d.ap())
            bvs_t = pp.tile([128, CCH], F32, tag="bvs")
            nc.sync.dma_start(
                bvs_t[:], bvs_d.ap().rearrange("(a p) b -> p (a b)", p=128))
            onesc_t = pp.tile([128, 1], F32R, tag="onesc")
            nc.sync.dma_start(onesc_t[:], onesc_d.ap())
            onesr_t = pp.tile([1, 128], F32R, tag="onesr")
            nc.sync.dma_start(onesr_t[:], onesr_d.ap())

            q_t = pp.tile([D, NQ], F32R, tag="q")
            k_t = pp.tile([D, N], F32R, tag="k")
            vt_t = pp.tile([128, NJT, C], F32R, tag="vt")

            # ---- phase A: projections ----
            # q[d, i] (only first NQ columns of x = this core's queries)
            for icq in range(NIC):
                ps = ps2.tile([128, IC], F32, tag="lg", name="pa_ps")
                for cc in range(CCH):
                    nc.tensor.matmul(
                        ps[:D, :], wq_t[:, cc, :],
                        x_t[cc][:, icq * IC:(icq + 1) * IC],
                        start=(cc == 0), stop=(cc == CCH - 1))
                nc.scalar.activation(
                    q_t[:, icq * IC:(icq + 1) * IC], ps[:D, :],
                    ACT_IDENT, bias=bq_t[:])
            # k[d, j] over all N columns
            for jc in range(N // IC):
                ps = ps2.tile([128, IC], F32, tag="lg", name="pa_ps")
                for cc in range(CCH):
                    nc.tensor.matmul(
                        ps[:D, :], wk_t[:, cc, :],
                        x_t[cc][:, jc * IC:(jc + 1) * IC],
                        start=(cc == 0), stop=(cc == CCH - 1))
                nc.scalar.activation(
                    k_t[:, jc * IC:(jc + 1) * IC], ps[:D, :],
                    ACT_IDENT, bias=bk_t[:])
            # vT[j, c] = sum_ch x[ch, j] * WvT'[ch, c]
            for jt in range(NJT):
                ps = ps2.tile([128, C], F32, tag="lg", name="pv_ps")
                for cc in range(CCH):
                    nc.tensor.matmul(
                        ps[:], x_t[cc][:, jt * 128:(jt + 1) * 128],
                        wv_t[:, cc, :],
                        start=(cc == 0), stop=(cc == CCH - 1))
                nc.scalar.activation(vt_t[:, jt, :], ps[:], ACT_COPY)

            # ---- phase B: attention, one query-chunk at a time ----
            for ic in range(NIC):
                av = [ps1.tile([128, IC], F32, tag=f"av{ct}", name=f"av{ct}")
                      for ct in range(CCH)]
                den = ps1.tile([1, IC], F32, tag="den", name="den")
                qs = q_t[:, ic * IC:(ic + 1) * IC]
                for jt in range(NJT):
                    lg = ps2.tile([128, IC], F32, tag="lg", name="lg")
                    nc.tensor.matmul(
                        lg[:], k_t[:, jt * 128:(jt + 1) * 128], qs,
                        start=True, stop=True)
                    ex = wp.tile([128, IC], F32R, tag="ex", name="ex")
                    nc.scalar.activation(ex[:], lg[:], ACT_EXP)
                    nc.tensor.matmul(
                        den[:], onesc_t[:], ex[:],
                        start=(jt == 0), stop=(jt == NJT - 1))
                    for ct in range(CCH):
                        nc.tensor.matmul(
                            av[ct][:], vt_t[:, jt, ct * 128:(ct + 1) * 128],
                            ex[:],
                            start=(jt == 0), stop=(jt == NJT - 1))
                # reciprocal of denominator, broadcast to 128 partitions
                den_sb = wp.tile([1, IC], F32, tag="den_sb", name="den_sb")
                nc.scalar.activation(den_sb[:], den[:], ACT_COPY)
                rec = wp.tile([1, IC], F32, tag="rec", name="rec")
                nc.vector.reciprocal(rec[:], den_sb[:])
                rec_r = wp.tile([1, IC], F32R, tag="rec_r", name="rec_r")
                nc.scalar.activation(rec_r[:], rec[:], ACT_COPY)
                bc = ps1.tile([128, IC], F32, tag="bc", name="bc")
                nc.tensor.matmul(bc[:], onesr_t[:], rec_r[:],
                                 start=True, stop=True)
                rdbc = fp.tile([128, IC], F32, tag="rdbc", name="rdbc")
                nc.scalar.activation(rdbc[:], bc[:], ACT_COPY)
                # out[c, i] = av[c, i] * rdbc[i] + bvs[c] + x[c, i]
                for ct in range(CCH):
                    t = fp.tile([128, IC], F32, tag="t", name="t")
                    nc.vector.tensor_mul(t[:], av[ct][:], rdbc[:])
                    o = fp.tile([128, IC], F32, tag="o", name="o")
                    nc.vector.scalar_tensor_tensor(
                        o[:], t[:], bvs_t[:, ct:ct + 1],
                        x_t[ct][:, ic * IC:(ic + 1) * IC].bitcast(F32),
                        op0=mybir.AluOpType.add, op1=mybir.AluOpType.add)
                    nc.sync.dma_start(
                        out_d.ap()[ct * 128:(ct + 1) * 128,
                                   ic * IC:(ic + 1) * IC], o[:])
    nc.compile()
    return nc


_RUNNER = None


def _get_runner():
    """Build the Bass program once and return a reusable jitted SPMD runner."""
    global _RUNNER
    if _RUNNER is not None:
        return _RUNNER

    import jax
    from jax.sharding import Mesh, PartitionSpec
    from jax.experimental.shard_map import shard_map
    from concourse import bass2jax

    nc = build()
    bass2jax.install_neuronx_cc_hook()

    partition_name = (nc.partition_id_tensor.name
                      if nc.partition_id_tensor else None)
    in_names = []
    out_names = []
    out_avals = []
    for alloc in nc.m.functions[0].allocations:
        if not isinstance(alloc, mybir.MemoryLocationSet):
            continue
        name = alloc.memorylocations[0].name
        if alloc.kind == "ExternalInput":
            if name != partition_name:
                in_names.append(name)
        elif alloc.kind == "ExternalOutput":
            out_names.append(name)
            out_avals.append(jax.core.ShapedArray(
                tuple(alloc.tensor_shape), mybir.dt.np(alloc.dtype)))
    n_params = len(in_names)
    n_outs = len(out_names)
    all_names = in_names + out_names
    if partition_name is not None:
        all_names = all_names + [partition_name]

    def _body(*args):
        operands = list(args)
        if partition_name is not None:
            operands.append(bass2jax.partition_id_tensor())
        outs = bass2jax._bass_exec_p.bind(
            *operands,
            out_avals=tuple(out_avals),
            in_names=tuple(all_names),
            out_names=tuple(out_names),
            lowering_input_output_aliases=(),
            sim_require_finite=True,
            sim_require_nnan=True,
            nc=nc,
        )
        return tuple(outs)

    devices = jax.devices()[:NCORES]
    mesh = Mesh(np.asarray(devices), ("core",))
    in_specs = (PartitionSpec("core"),) * (n_params + n_outs)
    out_specs = (PartitionSpec("core"),) * n_outs
    donate = tuple(range(n_params, n_params + n_outs))
    sharded = jax.jit(
        shard_map(_body, mesh=mesh, in_specs=in_specs, out_specs=out_specs,
                  check_rep=False),
        donate_argnums=donate, keep_unused=True)

    def run(in_maps):
        concat_in = [
            np.concatenate([np.asarray(m[name]) for m in in_maps], axis=0)
            for name in in_names
        ]
        concat_zeros = [
            np.zeros((NCORES * a.shape[0], *a.shape[1:]), a.dtype)
            for a in out_avals
        ]
        out_arrs = sharded(*concat_in, *concat_zeros)
        out_arrs = [np.asarray(a) for a in out_arrs]
        return [
            {name: out_arrs[i].reshape(NCORES, *out_avals[i].shape)[c]
             for i, name in enumerate(out_names)}
            for c in range(NCORES)
        ]

    _RUNNER = (run, nc)
    return _RUNNER


def make_in_maps(minibatch, Wq, bq, Wk, bk, Wv, bv, gamma):
    gamma0 = float(np.asarray(gamma).reshape(-1)[0])
    wqT = np.ascontiguousarray(np.asarray(Wq, np.float32).T)
    wkT = np.ascontiguousarray(np.asarray(Wk, np.float32).T)
    wvT = np.ascontiguousarray((gamma0 * np.asarray(Wv, np.float32)).T)
    bq2 = np.asarray(bq, np.float32).reshape(D, 1)
    bk2 = np.asarray(bk, np.float32).reshape(D, 1)
    bvs = (gamma0 * np.asarray(bv, np.float32)).reshape(C, 1)
    onesc = np.ones((128, 1), np.float32)
    onesr = np.ones((1, 128), np.float32)
    mb = np.asarray(minibatch, np.float32)
    in_maps = []
    for core in range(NCORES):
        b, h = divmod(core, 2)
        xb = mb[b]
        # roll so this core's query columns come first; key order is free
        xperm = np.ascontiguousarray(
            np.concatenate([xb[:, h * NQ:(h + 1) * NQ],
                            xb[:, (1 - h) * NQ:(2 - h) * NQ]], axis=1))
        in_maps.append(dict(x=xperm, wqT=wqT, wkT=wkT, wvT=wvT,
                            bq=bq2, bk=bk2, bvs=bvs,
                            onesc=onesc, onesr=onesr))
    return in_maps


def kernel(minibatch, Wq, bq, Wk, bk, Wv, bv, gamma):
    run, _ = _get_runner()
    in_maps = make_in_maps(minibatch, Wq, bq, Wk, bk, Wv, bv, gamma)
    results = run(in_maps)
    out = np.empty((B, C, N), np.float32)
    for core in range(NCORES):
        b, h = divmod(core, 2)
        out[b][:, h * NQ:(h + 1) * NQ] = results[core]["out"]
    return out
